# revision 9
# baseline (speedup 1.0000x reference)
"""Multi-head attention (B=4, T=2048, H=1024, nh=16) on 8 Trainium2 cores.

Sharding: core = (batch b, head-group g); 4 batches x 2 groups of 8 heads.
Each core computes Q^T/K^T projections for its 512 head-dims, the V
projection (shipped to HBM), and per head the softmax-weighted column
means cbar[s] = sum_t exp(scores[t,s])/denom[t].  Because the reference
takes mean over T before the output projection, the full [T,T]x[T,dh]
context matmul collapses: ctx_mean[d] = (1/T) sum_s cbar[s] V[s,d],
which the host finishes along with the (tiny) Wo projection.
"""

import numpy as np

B, T, C = 4, 2048, 1024
NH, DH = 16, 64
HLOC = 8          # heads per core
D = HLOC * DH     # 512 projection dims per core
N_CORES = 8

C_TILES = C // 128    # 8
T_TILES = T // 128    # 16
D_TILES = D // 128    # 4

_CACHE = {}
TRACE = False
TRACE_KWARGS = {}


def _build():
    import concourse.bass as bass
    import concourse.mybir as mybir
    import concourse.tile as tile
    from concourse import bacc

    f32 = mybir.dt.float32
    f32r = mybir.dt.float32r
    Exp = mybir.ActivationFunctionType.Exp

    nc = bacc.Bacc("TRN2", target_bir_lowering=False, debug=False,
                   num_devices=N_CORES)

    XT = nc.dram_tensor("xT", [C, T], f32, kind="ExternalInput").ap()
    WQT = nc.dram_tensor("wqT", [C, D], f32, kind="ExternalInput").ap()
    WKT = nc.dram_tensor("wkT", [C, D], f32, kind="ExternalInput").ap()
    WVT = nc.dram_tensor("wvT", [C, D], f32, kind="ExternalInput").ap()
    CBAR = nc.dram_tensor("cbar", [HLOC, T], f32, kind="ExternalOutput").ap()
    VOUT = nc.dram_tensor("vout", [T, D], f32, kind="ExternalOutput").ap()

    with tile.TileContext(nc) as tc, \
         nc.allow_low_precision("float32r tags carry full fp32 bits"):
        with tc.tile_pool(name="persist", bufs=1) as persist:
            # [d, t] layouts; d-tile dt at cols [dt*T, (dt+1)*T)
            qt_all = persist.tile([128, D_TILES * T], f32r)
            kt_all = persist.tile([128, D_TILES * T], f32r)

            # ---- load + projections (xT/W freed afterwards) ----
            with tc.tile_pool(name="load", bufs=1) as load, \
                 tc.tile_pool(name="proj_ps", bufs=4, space="PSUM") as proj_ps, \
                 tc.tile_pool(name="vstage", bufs=3) as vstage:
                xt_all = load.tile([128, C_TILES * T], f32r)
                for c in range(C_TILES):
                    nc.sync.dma_start(xt_all[:, c * T:(c + 1) * T],
                                      XT[c * 128:(c + 1) * 128, :].bitcast(f32r))
                wq_all = load.tile([128, C_TILES * D], f32r)
                wk_all = load.tile([128, C_TILES * D], f32r)
                wv_all = load.tile([128, C_TILES * D], f32r)
                for w_all, src in ((wq_all, WQT), (wk_all, WKT), (wv_all, WVT)):
                    for c in range(C_TILES):
                        nc.sync.dma_start(w_all[:, c * D:(c + 1) * D],
                                          src[c * 128:(c + 1) * 128, :].bitcast(f32r))

                # Q^T, K^T: out[d,t] accumulated over c
                for w_all, dst in ((wq_all, qt_all), (wk_all, kt_all)):
                    for dt_ in range(D_TILES):
                        for tb in range(4):            # t-blocks of 512
                            p = proj_ps.tile([128, 512], f32)
                            for c in range(C_TILES):
                                nc.tensor.matmul(
                                    p[:],
                                    w_all[:, c * D + dt_ * 128: c * D + (dt_ + 1) * 128],
                                    xt_all[:, c * T + tb * 512: c * T + tb * 512 + 512],
                                    start=(c == 0), stop=(c == C_TILES - 1))
                            nc.vector.tensor_copy(
                                dst[:, dt_ * T + tb * 512: dt_ * T + tb * 512 + 512],
                                p[:])

                # V: out[t, d] accumulated over c; straight to HBM
                for tt in range(T_TILES):
                    p = proj_ps.tile([128, 512], f32)
                    for c in range(C_TILES):
                        nc.tensor.matmul(
                            p[:],
                            xt_all[:, c * T + tt * 128: c * T + (tt + 1) * 128],
                            wv_all[:, c * D:(c + 1) * D],
                            start=(c == 0), stop=(c == C_TILES - 1))
                    vs = vstage.tile([128, D], f32)
                    nc.vector.tensor_copy(vs[:], p[:])
                    nc.sync.dma_start(VOUT[tt * 128:(tt + 1) * 128, :], vs[:])

            # ---- attention: per head, stream t-tiles ----
            with tc.tile_pool(name="score_ps", bufs=2, space="PSUM") as score_ps, \
                 tc.tile_pool(name="cbar_ps", bufs=1, space="PSUM") as cbar_ps, \
                 tc.tile_pool(name="wpool", bufs=3) as wpool, \
                 tc.tile_pool(name="small", bufs=4) as small, \
                 tc.tile_pool(name="stage", bufs=2) as stage_pool:
                for H in range(HLOC):
                    pair = H // 2
                    row0 = 64 * (H % 2)
                    cb = [cbar_ps.tile([1, 512], f32, tag=f"cb{j}", name=f"cb{j}")
                          for j in range(4)]
                    for tt in range(T_TILES):
                        qs = qt_all[row0:row0 + 64,
                                    pair * T + tt * 128: pair * T + (tt + 1) * 128]
                        sc = [score_ps.tile([128, 1024], f32, tag="sc", name=f"sc{i}")
                              for i in range(2)]
                        for i in range(2):
                            for j in range(2):
                                s_blk = i * 2 + j
                                nc.tensor.matmul(
                                    sc[i][:, j * 512:(j + 1) * 512],
                                    qs,
                                    kt_all[row0:row0 + 64,
                                           pair * T + s_blk * 512:
                                           pair * T + s_blk * 512 + 512],
                                    start=True, stop=True)
                        w = wpool.tile([128, T], f32r)
                        accs = small.tile([128, 2], f32, tag="accs")
                        for i in range(2):
                            nc.scalar.activation(
                                w[:, i * 1024:(i + 1) * 1024], sc[i][:], Exp,
                                scale=0.125, accum_out=accs[:, i:i + 1])
                        denom = small.tile([128, 1], f32, tag="denom")
                        nc.vector.tensor_add(denom[:], accs[:, 0:1], accs[:, 1:2])
                        r32 = small.tile([128, 1], f32, tag="r32")
                        nc.vector.reciprocal(r32[:], denom[:])
                        r = small.tile([128, 1], f32r, tag="r")
                        nc.vector.tensor_copy(r[:], r32[:])
                        for j in range(4):
                            nc.tensor.matmul(
                                cb[j][:], r[:], w[:, j * 512:(j + 1) * 512],
                                start=(tt == 0), stop=(tt == T_TILES - 1))
                    stg = stage_pool.tile([1, T], f32)
                    for j in range(4):
                        nc.vector.tensor_copy(stg[:, j * 512:(j + 1) * 512], cb[j][:])
                    nc.sync.dma_start(CBAR[H:H + 1, :], stg[:])

    nc.compile()
    return nc


def _setup_exec():
    """Build the Bass module and a cached jitted SPMD executor
    (mirrors concourse.bass2jax.run_bass_via_pjrt's multi-core path)."""
    import jax
    import concourse.mybir as mybir
    from concourse import bass2jax
    from jax.experimental.shard_map import shard_map
    from jax.sharding import Mesh, PartitionSpec

    nc = _build()
    bass2jax.install_neuronx_cc_hook()

    partition_name = (nc.partition_id_tensor.name
                      if nc.partition_id_tensor else None)
    in_names, out_names, out_avals, zero_shapes = [], [], [], []
    for alloc in nc.m.functions[0].allocations:
        if not isinstance(alloc, mybir.MemoryLocationSet):
            continue
        name = alloc.memorylocations[0].name
        if alloc.kind == "ExternalInput":
            if name != partition_name:
                in_names.append(name)
        elif alloc.kind == "ExternalOutput":
            shape = tuple(alloc.tensor_shape)
            dtype = mybir.dt.np(alloc.dtype)
            out_names.append(name)
            out_avals.append(jax.core.ShapedArray(shape, dtype))
            zero_shapes.append((shape, dtype))
    n_params = len(in_names)
    all_in_names = in_names + out_names
    if partition_name is not None:
        all_in_names = all_in_names + [partition_name]

    def _body(*args):
        operands = list(args)
        if partition_name is not None:
            operands.append(bass2jax.partition_id_tensor())
        outs = bass2jax._bass_exec_p.bind(
            *operands,
            out_avals=tuple(out_avals),
            in_names=tuple(all_in_names),
            out_names=tuple(out_names),
            lowering_input_output_aliases=(),
            sim_require_finite=True,
            sim_require_nnan=True,
            nc=nc,
        )
        return tuple(outs)

    devices = jax.devices()[:N_CORES]
    mesh = Mesh(np.asarray(devices), ("core",))
    n_outs = len(out_names)
    sharded = jax.jit(
        shard_map(_body, mesh=mesh,
                  in_specs=(PartitionSpec("core"),) * (n_params + n_outs),
                  out_specs=(PartitionSpec("core"),) * n_outs,
                  check_rep=False),
        donate_argnums=tuple(range(n_params, n_params + n_outs)),
        keep_unused=True,
    )

    from jax.sharding import NamedSharding
    shardings = NamedSharding(mesh, PartitionSpec("core"))

    def make_zeros():
        import jax.numpy as jnp
        return [
            jax.device_put(
                jnp.zeros((N_CORES * s[0], *s[1:]), d), shardings)
            for s, d in zero_shapes
        ]

    _CACHE.update(nc=nc, sharded=sharded, in_names=in_names,
                  out_names=out_names, out_avals=out_avals,
                  make_zeros=make_zeros, shardings=shardings)


def kernel(x, Wq, Wk, Wv, Wo, bo):
    import jax

    x = np.asarray(x, dtype=np.float32)
    Wq = np.asarray(Wq, dtype=np.float32)
    Wk = np.asarray(Wk, dtype=np.float32)
    Wv = np.asarray(Wv, dtype=np.float32)
    Wo = np.asarray(Wo, dtype=np.float32)
    bo = np.asarray(bo, dtype=np.float32)

    if "sharded" not in _CACHE:
        _setup_exec()

    in_maps = []
    for b in range(B):
        xtb = np.ascontiguousarray(x[b].T)            # [C, T]
        for g in range(2):
            rows = slice(g * D, (g + 1) * D)
            in_maps.append({
                "xT": xtb,
                "wqT": np.ascontiguousarray(Wq[rows, :].T),
                "wkT": np.ascontiguousarray(Wk[rows, :].T),
                "wvT": np.ascontiguousarray(Wv[rows, :].T),
            })

    concat_in = [
        np.concatenate([in_maps[c][name] for c in range(N_CORES)], axis=0)
        for name in _CACHE["in_names"]
    ]
    device_inputs = [jax.device_put(a, _CACHE["shardings"]) for a in concat_in]
    _CACHE["device_inputs"] = device_inputs

    out_arrs = _CACHE["sharded"](*device_inputs, *_CACHE["make_zeros"]())
    results = [
        {name: np.asarray(out_arrs[i]).reshape(N_CORES, *_CACHE["out_avals"][i].shape)[c]
         for i, name in enumerate(_CACHE["out_names"])}
        for c in range(N_CORES)
    ]

    ctx_mean = np.empty((B, C), dtype=np.float32)
    for core in range(N_CORES):
        b, g = divmod(core, 2)
        cbar = results[core]["cbar"]                  # [8, T]
        vout = results[core]["vout"]                  # [T, 512]
        v_r = vout.reshape(T, HLOC, DH)
        cm = np.einsum("hs,shd->hd", cbar, v_r, optimize=True) / np.float32(T)
        ctx_mean[b, g * D:(g + 1) * D] = cm.reshape(-1)

    return ctx_mean @ Wo.T + bo
